# revision 35
# baseline (speedup 1.0000x reference)
"""GCN message-passing kernel for Trainium2 (8 NeuronCores, Bass/Tile).

out = coef * relu(C_U * D^-1/2 A~^T D^-1/2 (x W^T + b)),  A~ = A + I

Strategy (dst-sharded, fully static SPMD program):
- Core c owns a 12,500-node dst range. Host deals dsts into 64-wide
  "windows" (LPT vector bin-packing + swap refinement for balance),
  buckets each core's edges by (window, src-segment), pads to static
  per-(window,seg) quotas (max over cores/windows, mult of 4) so all
  8 cores run one program.
- W commutes with aggregation: aggregate x rows (fp16) first, apply W
  once per output node afterwards.
- Gather calls are decoupled from the window/group structure: each
  src-segment stream is one long slot array; dma_gather calls of
  exactly 1024 idxs (994ns fixed overhead each) write 8-column
  per-call tiles. PE passes address (call, col%8) directly.
- Device: dma_gather pulls fp16 x rows (256B) from HBM by int16 index
  (4 src segments of 32768 rows); the self-loop stream is a host
  pre-permuted table read sequentially and kept resident in SBUF.
  DVE builds per-pass "value-hot" [128,64] matrices
  (iota==dstoff)*dis_src in fp16; PE contracts msgs^T @ vh into PSUM
  [128=D, 64=dst] per window; stage-2 matmul applies W^T; ACT fuses
  relu + coef*C_U*dis_dst scale; DMA out per group.
- Host unpermutes the window-ordered output rows.
"""

import sys
import types

import numpy as np


def _install_ntff_hook_bridge():
    """antenv.axon_hooks is missing from this image; bridge it so
    run_bass_kernel_spmd(trace=True) can profile. Harmless if unused."""
    if "antenv.axon_hooks" in sys.modules:
        return
    hooks = types.ModuleType("antenv.axon_hooks")
    hooks._HOOK = None

    def _get():
        if hooks._HOOK is None:
            try:
                from trn_agent_boot.trn_boot import _ntff_profile_via_ctypes

                hooks._HOOK = _ntff_profile_via_ctypes("/opt/axon/libaxon_pjrt.so")
            except Exception:
                hooks._HOOK = None
        return hooks._HOOK

    hooks.get_axon_ntff_profile_hook = _get
    hooks.set_axon_ntff_profile_hook = lambda h: setattr(hooks, "_HOOK", h)
    sys.modules["antenv.axon_hooks"] = hooks


_install_ntff_hook_bridge()

C_SIGMA = 2.0
C_U = 1.0
SEG = 32768  # dma_gather int16 index reach
W_WIN = 64  # dst window width (one-hot width)
N_CORES = 8
CALL = 1024  # idxs per dma_gather call (HW SWDGE ring limit)
TILE_COLS = CALL // 128  # 8 cols per gather-call tile


def _ceil(a, b):
    return (a + b - 1) // b


def _wrap16(idx, ncols):
    """[n] int16 -> [128, ncols] wrapped in 16 partitions, replicated x8."""
    n = idx.shape[0]
    out = np.zeros((16, ncols), dtype=np.int16)
    out[np.arange(n) % 16, np.arange(n) // 16] = idx
    return np.tile(out, (8, 1))


class _Prep:
    """Host-side sharding/preprocessing result."""


def _refine_windows(sd, win_of, pos_of, memb, caps, max_iters=40000):
    """Swap-refinement: reduce per-seg max window load toward caps.

    sd: [npc, nseg] per-node per-seg degree; win_of/pos_of: [npc];
    memb: [nwin*w_win]; caps: [nseg] target per-window loads.
    Mutates win_of/pos_of/memb in place.
    """
    nseg = sd.shape[1]
    nwin = memb.shape[0] // W_WIN
    loads = np.zeros((nwin, nseg), dtype=np.int64)
    for q in range(nseg):
        np.add.at(loads[:, q], win_of, sd[:, q])
    rng = np.random.default_rng(12345)
    caps = np.asarray(caps, dtype=np.int64)
    for _ in range(max_iters):
        over = loads - caps  # [nwin, nseg]
        wq = np.unravel_index(np.argmax(over), over.shape)
        w_hot, q_hot = int(wq[0]), int(wq[1])
        if over[w_hot, q_hot] <= 0:
            break  # all windows within caps
        # members of the hot window, heaviest in q_hot first
        hot_members = memb[w_hot * W_WIN : (w_hot + 1) * W_WIN]
        hot_members = hot_members[hot_members >= 0]
        hm = hot_members[np.argsort(-sd[hot_members, q_hot])][:12]
        # candidate cold windows: lowest load in q_hot
        cold_ws = np.argsort(loads[:, q_hot])[:12]
        best = None
        for d1 in hm:
            v1 = sd[d1]
            for w2 in cold_ws:
                if w2 == w_hot:
                    continue
                sl2 = memb[w2 * W_WIN : (w2 + 1) * W_WIN]
                c_members = sl2[sl2 >= 0]
                cur_pk = max((loads[w_hot] - caps).max(), (loads[w2] - caps).max())
                if len(c_members) < W_WIN:
                    # move d1 into the free slot of w2 (no swap partner)
                    nl1 = loads[w_hot] - v1
                    nl2 = loads[w2] + v1
                    new_pk = max((nl1 - caps).max(), (nl2 - caps).max())
                    if new_pk < cur_pk and (best is None or new_pk < best[0]):
                        best = (new_pk, d1, None, w2)
                if len(c_members) == 0:
                    continue
                # try the two lightest-in-q_hot members of w2
                cand2 = c_members[np.argsort(sd[c_members, q_hot])[:2]]
                for d2 in cand2:
                    v2 = sd[d2]
                    nl1 = loads[w_hot] - v1 + v2
                    nl2 = loads[w2] - v2 + v1
                    new_pk = max((nl1 - caps).max(), (nl2 - caps).max())
                    if new_pk < cur_pk and (best is None or new_pk < best[0]):
                        best = (new_pk, d1, d2, w2)
        if best is None:
            break
        _, d1, d2, w2 = best
        p1 = pos_of[d1]
        if d2 is None:
            # move: place d1 into w2's first free slot
            sl2 = memb[w2 * W_WIN : (w2 + 1) * W_WIN]
            p2 = int(np.argmax(sl2 < 0))
            memb[w_hot * W_WIN + p1] = -1
            memb[w2 * W_WIN + p2] = d1
            win_of[d1] = w2
            pos_of[d1] = p2
            loads[w_hot] -= sd[d1]
            loads[w2] += sd[d1]
        else:
            p2 = pos_of[d2]
            memb[w_hot * W_WIN + p1] = d2
            memb[w2 * W_WIN + p2] = d1
            win_of[d1], win_of[d2] = w2, w_hot
            pos_of[d1], pos_of[d2] = p2, p1
            loads[w_hot] += sd[d2] - sd[d1]
            loads[w2] += sd[d1] - sd[d2]
    return loads


def prepare(x, edge_index, W, b, n_cores=N_CORES, w_win=W_WIN, group=8):
    f16 = np.float16
    N, D = x.shape
    assert N % n_cores == 0
    npc = N // n_cores
    nwin = _ceil(npc, w_win)
    nwin = _ceil(nwin, 4) * 4
    nseg = _ceil(N, SEG)

    src = np.asarray(edge_index[0], dtype=np.int64)
    dst = np.asarray(edge_index[1], dtype=np.int64)
    deg = np.bincount(src, minlength=N).astype(np.float32) + 1.0
    dis = deg ** -0.5  # float32

    p = _Prep()
    p.N, p.D, p.npc, p.nwin, p.nseg = N, D, npc, nwin, nseg
    p.n_cores, p.w_win, p.group = n_cores, w_win, group
    p.coef = np.sqrt(C_SIGMA / D).astype(np.float32)
    p.x16 = np.ascontiguousarray(x.astype(f16))

    core_of = dst // npc
    dstloc = dst - core_of * npc

    # per-(node, segment) in-degree for balanced window packing
    segdeg = np.zeros((N, nseg), dtype=np.int64)
    np.add.at(segdeg, (dst, src // SEG), 1)

    # target per-window caps per seg (mult of 4); refined below
    seg_tot = np.zeros((n_cores, nseg), dtype=np.int64)
    np.add.at(seg_tot, (core_of, src // SEG), 1)
    caps = []
    for q in range(nseg):
        lo = _ceil(seg_tot[:, q].max(), nwin)  # per-window lower bound
        caps.append(max(16, _ceil(lo + 2, 4) * 4))

    p.win_members = []
    p.win_of = np.empty((n_cores, npc), dtype=np.int32)
    p.pos_of = np.empty((n_cores, npc), dtype=np.int32)
    realized = np.zeros(nseg, dtype=np.int64)
    for c in range(n_cores):
        sd = segdeg[c * npc : (c + 1) * npc]  # [npc, nseg]
        tot = sd.sum(axis=1)
        order = np.argsort(-tot, kind="stable").astype(np.int32)
        loads = np.zeros((nwin, nseg), dtype=np.float64)
        counts = np.zeros(nwin, dtype=np.int64)
        memb = -np.ones(nwin * w_win, dtype=np.int64)
        full_pen = np.zeros(nwin)
        win_of = p.win_of[c]
        pos_of = p.pos_of[c]
        for d in order:
            cand = (loads + sd[d]).max(axis=1) + full_pen
            w = int(np.argmin(cand))
            r = counts[w]
            counts[w] = r + 1
            if counts[w] >= w_win:
                full_pen[w] = 1e18
            loads[w] += sd[d]
            win_of[d] = w
            pos_of[d] = r
            memb[w * w_win + r] = d
        fl = _refine_windows(sd, win_of, pos_of, memb, caps)
        realized = np.maximum(realized, fl.max(axis=0))
        p.win_members.append(memb)

    # final quotas: refined caps, or realized max if refinement fell short
    p.quotas = [
        max(caps[q], int(_ceil(realized[q], 4) * 4)) for q in range(nseg)
    ]
    p.nstream = nseg  # gather streams (self is separate)

    # --- per-edge: core, window, dstoff, segment
    e_w = p.win_of[core_of, dstloc]
    e_off = p.pos_of[core_of, dstloc]
    e_q = (src // SEG).astype(np.int64)

    # --- stream geometry (same for all cores)
    # stream q: S = nwin*Q slots; padded to P (mult of 128); cols = P/128
    p.S = [nwin * Q for Q in p.quotas]
    p.P = [_ceil(s, 128) * 128 for s in p.S]
    p.cols = [pp // 128 for pp in p.P]
    # gather calls per stream: exactly CALL idxs, last partial
    p.calls = []  # (q, start, n) in window-progress order
    tmp = []
    for q in range(nseg):
        for st in range(0, p.P[q], CALL):
            n = min(CALL, p.P[q] - st)
            tmp.append((st / max(1, p.quotas[q]), q, st, n))
    tmp.sort()
    p.calls = [(q, st, n) for _, q, st, n in tmp]

    # --- pass schedule: per group, streams 0..nseg-1 then self
    # self stream: Q=64, slot w*64+i, col span within xself table
    p.selfQ = w_win
    p.ngroups = _ceil(nwin, group)
    p.group_sizes = [min(group, nwin - g * group) for g in range(p.ngroups)]

    def spans(w, Q):
        c0 = (w * Q) // 128
        c1 = ((w + 1) * Q - 1) // 128
        return range(c0, c1 + 1)

    # win_passes[w]: list of (q, col) with q==nseg meaning self stream
    p.win_passes = [[] for _ in range(nwin)]
    # vh metadata order: group-major, stream-major, window, col
    p.pass_order = []  # (g, q, w, col)
    p.gq_npas = [[0] * (nseg + 1) for _ in range(p.ngroups)]
    for g, gs in enumerate(p.group_sizes):
        for q in range(nseg + 1):
            Q = p.quotas[q] if q < nseg else p.selfQ
            for w in range(g * group, g * group + gs):
                for col in spans(w, Q):
                    p.pass_order.append((g, q, w, col))
                    p.win_passes[w].append((q, col))
                    p.gq_npas[g][q] += 1
    p.tot_pass = len(p.pass_order)
    p.colsmax = [max(p.gq_npas[g][q] for g in range(p.ngroups)) for q in range(nseg + 1)]

    # --- per-core slot fill + per-pass metadata
    p.idx_all = []  # [128, sum_q P_q/16] int16
    p.doff_all = []  # [128, tot_pass] f16
    p.disv_all = []  # [128, tot_pass] f16
    p.sd = []
    p.xself_perm = []
    p.idx_cols_tot = sum(pp // 16 for pp in p.P)

    for c in range(n_cores):
        m = core_of == c
        cw, coff, cq, csrc = e_w[m], e_off[m], e_q[m], src[m]
        memb = p.win_members[c]

        sl_doff = []
        sl_dis = []
        idx_cols = np.zeros((128, p.idx_cols_tot), dtype=np.int16)
        ic = 0
        for q in range(nseg):
            Q = p.quotas[q]
            S, P = p.S[q], p.P[q]
            nrows_q = min(N, (q + 1) * SEG) - q * SEG
            # spread pad reads across the segment (identical pad indices
            # serialize on one DRAM row)
            idx16 = ((np.arange(P, dtype=np.int64) * 7919) % nrows_q).astype(
                np.int16
            )
            doffv = -np.ones(P, dtype=np.float32)
            disv = np.zeros(P, dtype=np.float32)
            mq = cq == q
            wq, offq, srcq = cw[mq], coff[mq], csrc[mq]
            o = np.lexsort((srcq, wq))  # window-major, src-sorted within
            wq, offq, srcq = wq[o], offq[o], srcq[o]
            wcnt = np.bincount(wq, minlength=nwin)
            assert wcnt.max() <= Q, (c, q, wcnt.max(), Q)
            starts = np.concatenate([[0], np.cumsum(wcnt)[:-1]])
            rank = np.arange(len(wq)) - starts[wq]
            slot = wq * Q + rank
            idx16[slot] = (srcq - q * SEG).astype(np.int16)
            doffv[slot] = offq
            disv[slot] = dis[srcq]
            sl_doff.append(doffv)
            sl_dis.append(disv)
            ncol16 = P // 16
            idx_cols[:, ic : ic + ncol16] = _wrap16(idx16, ncol16)
            ic += ncol16
        assert ic == p.idx_cols_tot
        # self stream: host pre-permutes the core's x slice into wrapped
        # window order -> sequential DMA, resident in SBUF (no gather).
        S = nwin * w_win
        doffv = -np.ones(S, dtype=np.float32)
        disv = np.zeros(S, dtype=np.float32)
        real = memb >= 0
        slots = np.arange(S)[real]
        nodes = memb[real]
        doffv[slots] = slots % w_win
        disv[slots] = dis[c * npc + nodes]
        sl_doff.append(doffv)
        sl_dis.append(disv)
        xsp = np.zeros((128, S // 128, D), dtype=f16)
        xsp[slots % 128, slots // 128] = p.x16[c * npc + nodes]
        p.xself_perm.append(xsp)

        # pass metadata in pass_order
        doff_cols = np.empty((128, p.tot_pass), dtype=f16)
        disv_cols = np.empty((128, p.tot_pass), dtype=f16)
        for i, (g, q, w, col) in enumerate(p.pass_order):
            Q = p.quotas[q] if q < nseg else p.selfQ
            s0 = col * 128
            sl = np.arange(s0, s0 + 128)
            inw = (sl >= w * Q) & (sl < (w + 1) * Q)
            if q < nseg:
                inw &= sl < p.S[q]
            dv = np.full(128, -1.0, dtype=np.float32)
            vv = np.zeros(128, dtype=np.float32)
            gsl = sl[inw]
            dv[inw] = sl_doff[q][gsl]
            vv[inw] = sl_dis[q][gsl]
            doff_cols[:, i] = dv.astype(f16)
            disv_cols[:, i] = vv.astype(f16)
        p.idx_all.append(idx_cols)
        p.doff_all.append(doff_cols)
        p.disv_all.append(disv_cols)

        sdv = np.zeros((w_win, nwin), dtype=np.float32)
        nodes_per_win = memb.reshape(nwin, w_win)
        for w in range(nwin):
            mm = nodes_per_win[w] >= 0
            sdv[mm, w] = (
                p.coef * C_U * dis[c * npc + nodes_per_win[w][mm]]
            ).astype(np.float32)
        p.sd.append(sdv)

    # iota-expanded constant [128, w_win, max colsmax] f16
    cm = max(p.colsmax)
    io = np.broadcast_to(
        np.arange(w_win, dtype=np.float32)[None, :, None], (128, w_win, cm)
    )
    p.iota = np.ascontiguousarray(io.astype(f16))
    p.iota_cm = cm
    p.WT = np.ascontiguousarray(np.asarray(W, dtype=np.float32).T)
    p.b = np.asarray(b, dtype=np.float32)
    p.bias_nonzero = bool(np.any(p.b != 0))
    if p.bias_nonzero:
        sb = np.zeros((n_cores, nwin * w_win), dtype=np.float32)
        np.add.at(sb, (core_of, e_w * w_win + e_off), dis[src])
        for c in range(n_cores):
            memb = p.win_members[c]
            real = memb >= 0
            slots = np.arange(nwin * w_win)[real]
            sb[c, slots] += dis[c * npc + memb[real]]
        p.sb = sb.reshape(n_cores, 1, nwin * w_win)
    return p


def build_program(p, gbufs=10, scratch=32768):
    import concourse.bacc as bacc
    import concourse.mybir as mybir
    import concourse.tile as tile

    f32, f16i, i16 = mybir.dt.float32, mybir.dt.float16, mybir.dt.int16
    D, nwin, group = p.D, p.nwin, p.group
    nseg = p.nseg

    nc = bacc.Bacc(
        "TRN2",
        target_bir_lowering=False,
        debug=False,
        num_swdge_queues=4,
        dynamic_dma_scratch_size=scratch,
    )
    x_d = nc.dram_tensor("x", [p.N, D], f16i, kind="ExternalInput")
    xself_d = nc.dram_tensor(
        "xself", [128, p.nwin * p.w_win // 128, D], f16i, kind="ExternalInput"
    )
    wt_d = nc.dram_tensor("wt", [D, D], f32, kind="ExternalInput")
    iota_d = nc.dram_tensor(
        "iota", [128, p.w_win, p.iota_cm], f16i, kind="ExternalInput"
    )
    idx_d = nc.dram_tensor("idx", [128, p.idx_cols_tot], i16, kind="ExternalInput")
    doff_d = nc.dram_tensor("doff", [128, p.tot_pass], f16i, kind="ExternalInput")
    disv_d = nc.dram_tensor("disv", [128, p.tot_pass], f16i, kind="ExternalInput")
    sd_d = nc.dram_tensor("sd", [p.w_win, nwin], f32, kind="ExternalInput")
    if p.bias_nonzero:
        sb_d = nc.dram_tensor("sb", [1, nwin * p.w_win], f32, kind="ExternalInput")
        b_d = nc.dram_tensor("b", [1, D], f32, kind="ExternalInput")
    out_d = nc.dram_tensor("out", [p.w_win, nwin, D], f16i, kind="ExternalOutput")

    segs = []
    for q in range(nseg):
        lo = q * SEG
        hi = min(p.N, lo + SEG)
        segs.append(x_d[lo:hi, :])

    # per-stream idx base col (16-wrapped)
    idx_base = []
    ib = 0
    for q in range(nseg):
        idx_base.append(ib)
        ib += p.P[q] // 16

    # vh metadata base per (g, q)
    vh_base = {}
    pb = 0
    for g in range(p.ngroups):
        for q in range(nseg + 1):
            vh_base[(g, q)] = pb
            pb += p.gq_npas[g][q]
    assert pb == p.tot_pass

    with tile.TileContext(nc) as tc:
        with (
            tc.tile_pool(name="const", bufs=1) as constp,
            tc.tile_pool(name="gbuf", bufs=1) as gbufp,
            tc.tile_pool(name="vh", bufs=2) as vhp,
            tc.tile_pool(name="aggx", bufs=3) as aggxp,
            tc.tile_pool(name="outsb", bufs=2) as outp,
            tc.tile_pool(name="ps1", bufs=4, space="PSUM") as ps1p,
            tc.tile_pool(name="ps2", bufs=2, space="PSUM") as ps2p,
        ):
            # idx: separate tiles per (stream, quarter) so the first gathers
            # only wait on the first ~800KB slice, not the whole 3.3MB table.
            # Slice boundaries are multiples of 64 cols (= one 1024-idx call)
            # so a call's idx slice never straddles tiles.
            idx_tiles = []  # per stream: list of (start_col, ncols, tile)
            for q in range(nseg):
                cq = p.P[q] // 16
                sl = _ceil(_ceil(cq, 4), 64) * 64
                tl = []
                for s in range(0, cq, sl):
                    e = min(cq, s + sl)
                    t = constp.tile([128, e - s], i16, tag=f"idx{q}_{s // sl}")
                    tl.append((s, e - s, t))
                idx_tiles.append(tl)
            idx_loaded = set()

            def _load_idx(q, k):
                if k >= len(idx_tiles[q]) or (q, k) in idx_loaded:
                    return
                idx_loaded.add((q, k))
                a, n, t = idx_tiles[q][k]
                nc.sync.dma_start(
                    t[:], idx_d[:, idx_base[q] + a : idx_base[q] + a + n]
                )

            # interleaved per-stream slices, head of each stream first
            for s in range(4):
                for q in range(nseg):
                    _load_idx(q, s)
            # meta tables go on the Scalar engine's HWDGE queue so they
            # don't queue behind the idx loads. iota + the first doff/disv
            # quarter go first so DVE can start building vh immediately.
            iota_sb = constp.tile([128, p.w_win, p.iota_cm], f16i, tag="iota")
            nc.scalar.dma_start(iota_sb[:], iota_d[:])
            gquart = _ceil(p.ngroups, 4)
            meta_tiles = []  # (pass_start, npas, doff_tile, disv_tile)
            for s in range(0, p.ngroups, gquart):
                ge = min(p.ngroups, s + gquart)
                a = vh_base[(s, 0)]
                e = vh_base[(ge, 0)] if ge < p.ngroups else p.tot_pass
                dt_ = constp.tile([128, e - a], f16i, tag=f"doff{s}")
                vt_ = constp.tile([128, e - a], f16i, tag=f"disv{s}")
                nc.scalar.dma_start(dt_[:], doff_d[:, a:e])
                nc.scalar.dma_start(vt_[:], disv_d[:, a:e])
                meta_tiles.append((a, e - a, dt_, vt_))
            wt32 = constp.tile([D, D], f32, tag="wt32")
            nc.scalar.dma_start(wt32[:], wt_d[:])
            wt16 = constp.tile([D, D], f16i, tag="wt16")
            nc.scalar.copy(wt16[:], wt32[:])
            sd_sb = constp.tile([p.w_win, nwin], f32, tag="sd")
            nc.scalar.dma_start(sd_sb[:], sd_d[:])
            xcols = p.nwin * p.w_win // 128
            xquart = _ceil(_ceil(xcols, 4), 1)
            xself_tiles = []  # (col_start, ncols, tile)
            for s in range(0, xcols, xquart):
                e = min(xcols, s + xquart)
                xt = constp.tile([128, e - s, D], f16i, tag=f"xself{s}")
                nc.scalar.dma_start(xt[:], xself_d[:, s:e, :])
                xself_tiles.append((s, e - s, xt))

            def _meta(base, npas):
                for a, n, dt_, vt_ in meta_tiles:
                    if a <= base and base + npas <= a + n:
                        return dt_[:, base - a : base - a + npas], vt_[
                            :, base - a : base - a + npas
                        ]
                raise AssertionError((base, npas))

            def _xself(col):
                for s, n, xt in xself_tiles:
                    if s <= col < s + n:
                        return xt[:, col - s, :]
                raise AssertionError(col)
            if p.bias_nonzero:
                sb_sb = constp.tile([1, nwin * p.w_win], f32, tag="sb")
                nc.sync.dma_start(sb_sb[:], sb_d[:])
                b32 = constp.tile([1, D], f32, tag="b32")
                nc.sync.dma_start(b32[:], b_d[:])
                b16 = constp.tile([1, D], f16i, tag="b16")
                nc.scalar.copy(b16[:], b32[:])
                sbrow16 = constp.tile([1, nwin * p.w_win], f16i, tag="sbw16")
                nc.scalar.copy(sbrow16[:], sb_sb[:])

            # gather calls: per-call tiles, issued in window-progress order
            gtiles = [{} for _ in range(nseg)]  # q -> {tile_index: handle}
            rot = [gbufs if p.P[q] > 8 * CALL else 2 for q in range(nseg)]
            for ci, (q, st, n) in enumerate(p.calls):
                t = st // CALL
                if t not in gtiles[q]:
                    gt = gbufp.tile(
                        [128, TILE_COLS, D], f16i, tag=f"g{q}_{t % rot[q]}"
                    )
                    gtiles[q][t] = gt
                gt = gtiles[q][t]
                lo = (st % CALL) // 128
                c0 = st // 16
                islice = None
                for k, (a, ncols, it) in enumerate(idx_tiles[q]):
                    if a <= c0 < a + ncols:
                        islice = it[:, c0 - a : c0 - a + n // 16]
                        break
                nc.gpsimd.dma_gather(
                    gt[:, lo : lo + n // 128, :],
                    segs[q],
                    islice,
                    n,
                    n,
                    D,
                    queue_num=ci % 4,
                )

            # vh builds + window passes, group-major
            for g, gs in enumerate(p.group_sizes):
                w0 = g * group
                vts = []
                for q in range(nseg + 1):
                    npas = p.gq_npas[g][q]
                    base = vh_base[(g, q)]
                    vt = vhp.tile([128, p.w_win, p.colsmax[q]], f16i, tag=f"v{q}")
                    doff_ap, disv_ap = _meta(base, npas)

                    def _bcast(ap2d, n=npas):
                        return ap2d.rearrange(
                            "p (o c) -> p o c", o=1
                        ).broadcast_to([128, p.w_win, n])

                    nc.vector.tensor_tensor(
                        vt[:, :, :npas],
                        iota_sb[:, :, :npas],
                        _bcast(doff_ap),
                        mybir.AluOpType.is_equal,
                    )
                    nc.vector.tensor_tensor(
                        vt[:, :, :npas],
                        vt[:, :, :npas],
                        _bcast(disv_ap),
                        mybir.AluOpType.mult,
                    )
                    vts.append(vt)

                out_sb = outp.tile([p.w_win, gs, D], f16i, tag="out")
                pass_ctr = [0] * (nseg + 1)
                for wl in range(gs):
                    w = w0 + wl
                    ps1 = ps1p.tile([D, p.w_win], f32, tag="ps1")
                    plist = p.win_passes[w]
                    for k, (q, col) in enumerate(plist):
                        if q < nseg:
                            ms = gtiles[q][col // TILE_COLS][
                                :, col % TILE_COLS, :
                            ]
                        else:
                            ms = _xself(col)
                        pl = pass_ctr[q]
                        pass_ctr[q] += 1
                        nc.tensor.matmul(
                            ps1[:, :],
                            ms,
                            vts[q][:, :, pl],
                            start=(k == 0),
                            stop=(k == len(plist) - 1),
                        )
                    ag = aggxp.tile([D, p.w_win], f16i, tag="ag")
                    nc.scalar.copy(ag[:], ps1[:])
                    ps2 = ps2p.tile([p.w_win, D], f32, tag="ps2")
                    nc.tensor.matmul(
                        ps2[:, :],
                        ag[:, :],
                        wt16[:, :],
                        start=True,
                        stop=not p.bias_nonzero,
                    )
                    if p.bias_nonzero:
                        nc.tensor.matmul(
                            ps2[:, :],
                            sbrow16[:, w * p.w_win : (w + 1) * p.w_win],
                            b16[:, :],
                            start=False,
                            stop=True,
                        )
                    nc.scalar.activation(
                        out_sb[:, wl, :],
                        ps2[:, :],
                        mybir.ActivationFunctionType.Relu,
                        scale=sd_sb[:, w : w + 1],
                    )
                nc.sync.dma_start(out_d[:, w0 : w0 + gs, :], out_sb[:])
    nc.compile()
    return nc


def _unshard(p, outs):
    N, D = p.N, p.D
    res = np.empty((N, D), dtype=np.float32)
    for c in range(p.n_cores):
        o = (
            np.asarray(outs[c])
            .astype(np.float32)
            .transpose(1, 0, 2)
            .reshape(p.nwin * p.w_win, D)
        )
        memb = p.win_members[c]
        real = memb >= 0
        res[c * p.npc + memb[real]] = o[real]
    return res


def _in_maps(p):
    maps = []
    for c in range(p.n_cores):
        m = {
            "x": p.x16,
            "xself": p.xself_perm[c],
            "wt": p.WT,
            "iota": p.iota,
            "idx": p.idx_all[c],
            "doff": p.doff_all[c],
            "disv": p.disv_all[c],
            "sd": p.sd[c],
        }
        if p.bias_nonzero:
            m["sb"] = p.sb[c]
            m["b"] = p.b.reshape(1, -1)
        maps.append(m)
    return maps


def kernel(x, edge_index, W, b):
    from concourse.bass_utils import run_bass_kernel_spmd

    x = np.asarray(x, dtype=np.float32)
    W = np.asarray(W, dtype=np.float32)
    b = np.asarray(b, dtype=np.float32)
    p = prepare(x, edge_index, W, b)
    nc = build_program(p)
    res = run_bass_kernel_spmd(nc, _in_maps(p), core_ids=list(range(p.n_cores)))
    outs = [r["out"] for r in res.results]
    return _unshard(p, outs)


# revision 36
# speedup vs baseline: 1.0201x; 1.0201x over previous
"""GCN message-passing kernel for Trainium2 (8 NeuronCores, Bass/Tile).

out = coef * relu(C_U * D^-1/2 A~^T D^-1/2 (x W^T + b)),  A~ = A + I

Strategy (dst-sharded, fully static SPMD program):
- Core c owns a 12,500-node dst range. Host deals dsts into 64-wide
  "windows" (LPT vector bin-packing + swap refinement for balance),
  buckets each core's edges by (window, src-segment), pads to static
  per-(window,seg) quotas (max over cores/windows, mult of 4) so all
  8 cores run one program.
- W commutes with aggregation: aggregate x rows (fp16) first, apply W
  once per output node afterwards.
- Gather calls are decoupled from the window/group structure: each
  src-segment stream is one long slot array; dma_gather calls of
  exactly 1024 idxs (994ns fixed overhead each) write 8-column
  per-call tiles. PE passes address (call, col%8) directly.
- Device: dma_gather pulls fp16 x rows (256B) from HBM by int16 index
  (4 src segments of 32768 rows); the self-loop stream is a host
  pre-permuted table read sequentially and kept resident in SBUF.
  DVE builds per-pass "value-hot" [128,64] matrices
  (iota==dstoff)*dis_src in fp16; PE contracts msgs^T @ vh into PSUM
  [128=D, 64=dst] per window; stage-2 matmul applies W^T; ACT fuses
  relu + coef*C_U*dis_dst scale; DMA out per group.
- Host unpermutes the window-ordered output rows.
"""

import sys
import types

import numpy as np


def _install_ntff_hook_bridge():
    """antenv.axon_hooks is missing from this image; bridge it so
    run_bass_kernel_spmd(trace=True) can profile. Harmless if unused."""
    if "antenv.axon_hooks" in sys.modules:
        return
    hooks = types.ModuleType("antenv.axon_hooks")
    hooks._HOOK = None

    def _get():
        if hooks._HOOK is None:
            try:
                from trn_agent_boot.trn_boot import _ntff_profile_via_ctypes

                hooks._HOOK = _ntff_profile_via_ctypes("/opt/axon/libaxon_pjrt.so")
            except Exception:
                hooks._HOOK = None
        return hooks._HOOK

    hooks.get_axon_ntff_profile_hook = _get
    hooks.set_axon_ntff_profile_hook = lambda h: setattr(hooks, "_HOOK", h)
    sys.modules["antenv.axon_hooks"] = hooks


_install_ntff_hook_bridge()

C_SIGMA = 2.0
C_U = 1.0
SEG = 32768  # dma_gather int16 index reach
W_WIN = 64  # dst window width (one-hot width)
N_CORES = 8
CALL = 1024  # idxs per dma_gather call (HW SWDGE ring limit)
TILE_COLS = CALL // 128  # 8 cols per gather-call tile


def _ceil(a, b):
    return (a + b - 1) // b


def _wrap16(idx, ncols):
    """[n] int16 -> [128, ncols] wrapped in 16 partitions, replicated x8."""
    n = idx.shape[0]
    out = np.zeros((16, ncols), dtype=np.int16)
    out[np.arange(n) % 16, np.arange(n) // 16] = idx
    return np.tile(out, (8, 1))


class _Prep:
    """Host-side sharding/preprocessing result."""


def _refine_windows(sd, win_of, pos_of, memb, caps, max_iters=40000):
    """Swap-refinement: reduce per-seg max window load toward caps.

    sd: [npc, nseg] per-node per-seg degree; win_of/pos_of: [npc];
    memb: [nwin*w_win]; caps: [nseg] target per-window loads.
    Mutates win_of/pos_of/memb in place.
    """
    nseg = sd.shape[1]
    nwin = memb.shape[0] // W_WIN
    loads = np.zeros((nwin, nseg), dtype=np.int64)
    for q in range(nseg):
        np.add.at(loads[:, q], win_of, sd[:, q])
    rng = np.random.default_rng(12345)
    caps = np.asarray(caps, dtype=np.int64)
    for _ in range(max_iters):
        over = loads - caps  # [nwin, nseg]
        wq = np.unravel_index(np.argmax(over), over.shape)
        w_hot, q_hot = int(wq[0]), int(wq[1])
        if over[w_hot, q_hot] <= 0:
            break  # all windows within caps
        # members of the hot window, heaviest in q_hot first
        hot_members = memb[w_hot * W_WIN : (w_hot + 1) * W_WIN]
        hot_members = hot_members[hot_members >= 0]
        hm = hot_members[np.argsort(-sd[hot_members, q_hot])][:12]
        # candidate cold windows: lowest load in q_hot
        cold_ws = np.argsort(loads[:, q_hot])[:12]
        best = None
        for d1 in hm:
            v1 = sd[d1]
            for w2 in cold_ws:
                if w2 == w_hot:
                    continue
                sl2 = memb[w2 * W_WIN : (w2 + 1) * W_WIN]
                c_members = sl2[sl2 >= 0]
                cur_pk = max((loads[w_hot] - caps).max(), (loads[w2] - caps).max())
                if len(c_members) < W_WIN:
                    # move d1 into the free slot of w2 (no swap partner)
                    nl1 = loads[w_hot] - v1
                    nl2 = loads[w2] + v1
                    new_pk = max((nl1 - caps).max(), (nl2 - caps).max())
                    if new_pk < cur_pk and (best is None or new_pk < best[0]):
                        best = (new_pk, d1, None, w2)
                if len(c_members) == 0:
                    continue
                # try the two lightest-in-q_hot members of w2
                cand2 = c_members[np.argsort(sd[c_members, q_hot])[:2]]
                for d2 in cand2:
                    v2 = sd[d2]
                    nl1 = loads[w_hot] - v1 + v2
                    nl2 = loads[w2] - v2 + v1
                    new_pk = max((nl1 - caps).max(), (nl2 - caps).max())
                    if new_pk < cur_pk and (best is None or new_pk < best[0]):
                        best = (new_pk, d1, d2, w2)
        if best is None:
            break
        _, d1, d2, w2 = best
        p1 = pos_of[d1]
        if d2 is None:
            # move: place d1 into w2's first free slot
            sl2 = memb[w2 * W_WIN : (w2 + 1) * W_WIN]
            p2 = int(np.argmax(sl2 < 0))
            memb[w_hot * W_WIN + p1] = -1
            memb[w2 * W_WIN + p2] = d1
            win_of[d1] = w2
            pos_of[d1] = p2
            loads[w_hot] -= sd[d1]
            loads[w2] += sd[d1]
        else:
            p2 = pos_of[d2]
            memb[w_hot * W_WIN + p1] = d2
            memb[w2 * W_WIN + p2] = d1
            win_of[d1], win_of[d2] = w2, w_hot
            pos_of[d1], pos_of[d2] = p2, p1
            loads[w_hot] += sd[d2] - sd[d1]
            loads[w2] += sd[d1] - sd[d2]
    return loads


def prepare(x, edge_index, W, b, n_cores=N_CORES, w_win=W_WIN, group=8):
    f16 = np.float16
    N, D = x.shape
    assert N % n_cores == 0
    npc = N // n_cores
    nwin = _ceil(npc, w_win)
    nwin = _ceil(nwin, 4) * 4
    nseg = _ceil(N, SEG)

    src = np.asarray(edge_index[0], dtype=np.int64)
    dst = np.asarray(edge_index[1], dtype=np.int64)
    deg = np.bincount(src, minlength=N).astype(np.float32) + 1.0
    dis = deg ** -0.5  # float32

    p = _Prep()
    p.N, p.D, p.npc, p.nwin, p.nseg = N, D, npc, nwin, nseg
    p.n_cores, p.w_win, p.group = n_cores, w_win, group
    p.coef = np.sqrt(C_SIGMA / D).astype(np.float32)
    p.x16 = np.ascontiguousarray(x.astype(f16))

    core_of = dst // npc
    dstloc = dst - core_of * npc

    # per-(node, segment) in-degree for balanced window packing
    segdeg = np.zeros((N, nseg), dtype=np.int64)
    np.add.at(segdeg, (dst, src // SEG), 1)

    # target per-window caps per seg (mult of 4); refined below
    seg_tot = np.zeros((n_cores, nseg), dtype=np.int64)
    np.add.at(seg_tot, (core_of, src // SEG), 1)
    caps = []
    for q in range(nseg):
        lo = _ceil(seg_tot[:, q].max(), nwin)  # per-window lower bound
        caps.append(max(16, _ceil(lo + 2, 4) * 4))

    p.win_members = []
    p.win_of = np.empty((n_cores, npc), dtype=np.int32)
    p.pos_of = np.empty((n_cores, npc), dtype=np.int32)
    realized = np.zeros(nseg, dtype=np.int64)
    for c in range(n_cores):
        sd = segdeg[c * npc : (c + 1) * npc]  # [npc, nseg]
        tot = sd.sum(axis=1)
        order = np.argsort(-tot, kind="stable").astype(np.int32)
        loads = np.zeros((nwin, nseg), dtype=np.float64)
        counts = np.zeros(nwin, dtype=np.int64)
        memb = -np.ones(nwin * w_win, dtype=np.int64)
        full_pen = np.zeros(nwin)
        win_of = p.win_of[c]
        pos_of = p.pos_of[c]
        for d in order:
            cand = (loads + sd[d]).max(axis=1) + full_pen
            w = int(np.argmin(cand))
            r = counts[w]
            counts[w] = r + 1
            if counts[w] >= w_win:
                full_pen[w] = 1e18
            loads[w] += sd[d]
            win_of[d] = w
            pos_of[d] = r
            memb[w * w_win + r] = d
        fl = _refine_windows(sd, win_of, pos_of, memb, caps)
        realized = np.maximum(realized, fl.max(axis=0))
        p.win_members.append(memb)

    # final quotas: refined caps, or realized max if refinement fell short
    p.quotas = [
        max(caps[q], int(_ceil(realized[q], 4) * 4)) for q in range(nseg)
    ]
    p.nstream = nseg  # gather streams (self is separate)

    # --- per-edge: core, window, dstoff, segment
    e_w = p.win_of[core_of, dstloc]
    e_off = p.pos_of[core_of, dstloc]
    e_q = (src // SEG).astype(np.int64)

    # --- stream geometry (same for all cores)
    # stream q: S = nwin*Q slots; padded to P (mult of 128); cols = P/128
    p.S = [nwin * Q for Q in p.quotas]
    p.P = [_ceil(s, 128) * 128 for s in p.S]
    p.cols = [pp // 128 for pp in p.P]
    # gather calls per stream: exactly CALL idxs, last partial
    p.calls = []  # (q, start, n) in window-progress order
    tmp = []
    for q in range(nseg):
        for st in range(0, p.P[q], CALL):
            n = min(CALL, p.P[q] - st)
            tmp.append((st / max(1, p.quotas[q]), q, st, n))
    tmp.sort()
    p.calls = [(q, st, n) for _, q, st, n in tmp]

    # --- pass schedule: per group, streams 0..nseg-1 then self
    # self stream: Q=64, slot w*64+i, col span within xself table
    p.selfQ = w_win
    p.ngroups = _ceil(nwin, group)
    p.group_sizes = [min(group, nwin - g * group) for g in range(p.ngroups)]

    def spans(w, Q):
        c0 = (w * Q) // 128
        c1 = ((w + 1) * Q - 1) // 128
        return range(c0, c1 + 1)

    # win_passes[w]: list of (q, col) with q==nseg meaning self stream
    p.win_passes = [[] for _ in range(nwin)]
    # vh metadata order: group-major, stream-major, window, col
    p.pass_order = []  # (g, q, w, col)
    p.gq_npas = [[0] * (nseg + 1) for _ in range(p.ngroups)]
    for g, gs in enumerate(p.group_sizes):
        for q in range(nseg + 1):
            Q = p.quotas[q] if q < nseg else p.selfQ
            for w in range(g * group, g * group + gs):
                for col in spans(w, Q):
                    p.pass_order.append((g, q, w, col))
                    p.win_passes[w].append((q, col))
                    p.gq_npas[g][q] += 1
    p.tot_pass = len(p.pass_order)
    p.colsmax = [max(p.gq_npas[g][q] for g in range(p.ngroups)) for q in range(nseg + 1)]

    # --- per-core slot fill + per-pass metadata
    p.idx_all = []  # [128, sum_q P_q/16] int16
    p.doff_all = []  # [128, tot_pass] f16
    p.disv_all = []  # [128, tot_pass] f16
    p.sd = []
    p.xself_perm = []
    p.idx_cols_tot = sum(pp // 16 for pp in p.P)

    for c in range(n_cores):
        m = core_of == c
        cw, coff, cq, csrc = e_w[m], e_off[m], e_q[m], src[m]
        memb = p.win_members[c]

        sl_doff = []
        sl_dis = []
        idx_cols = np.zeros((128, p.idx_cols_tot), dtype=np.int16)
        ic = 0
        for q in range(nseg):
            Q = p.quotas[q]
            S, P = p.S[q], p.P[q]
            nrows_q = min(N, (q + 1) * SEG) - q * SEG
            # spread pad reads across the segment (identical pad indices
            # serialize on one DRAM row)
            idx16 = ((np.arange(P, dtype=np.int64) * 7919) % nrows_q).astype(
                np.int16
            )
            doffv = -np.ones(P, dtype=np.float32)
            disv = np.zeros(P, dtype=np.float32)
            mq = cq == q
            wq, offq, srcq = cw[mq], coff[mq], csrc[mq]
            o = np.lexsort((srcq, wq))  # window-major, src-sorted within
            wq, offq, srcq = wq[o], offq[o], srcq[o]
            wcnt = np.bincount(wq, minlength=nwin)
            assert wcnt.max() <= Q, (c, q, wcnt.max(), Q)
            starts = np.concatenate([[0], np.cumsum(wcnt)[:-1]])
            rank = np.arange(len(wq)) - starts[wq]
            slot = wq * Q + rank
            idx16[slot] = (srcq - q * SEG).astype(np.int16)
            doffv[slot] = offq
            disv[slot] = dis[srcq]
            sl_doff.append(doffv)
            sl_dis.append(disv)
            ncol16 = P // 16
            idx_cols[:, ic : ic + ncol16] = _wrap16(idx16, ncol16)
            ic += ncol16
        assert ic == p.idx_cols_tot
        # self stream: host pre-permutes the core's x slice into wrapped
        # window order -> sequential DMA, resident in SBUF (no gather).
        S = nwin * w_win
        doffv = -np.ones(S, dtype=np.float32)
        disv = np.zeros(S, dtype=np.float32)
        real = memb >= 0
        slots = np.arange(S)[real]
        nodes = memb[real]
        doffv[slots] = slots % w_win
        disv[slots] = dis[c * npc + nodes]
        sl_doff.append(doffv)
        sl_dis.append(disv)
        xsp = np.zeros((128, S // 128, D), dtype=f16)
        xsp[slots % 128, slots // 128] = p.x16[c * npc + nodes]
        p.xself_perm.append(xsp)

        # pass metadata in pass_order
        doff_cols = np.empty((128, p.tot_pass), dtype=f16)
        disv_cols = np.empty((128, p.tot_pass), dtype=f16)
        for i, (g, q, w, col) in enumerate(p.pass_order):
            Q = p.quotas[q] if q < nseg else p.selfQ
            s0 = col * 128
            sl = np.arange(s0, s0 + 128)
            inw = (sl >= w * Q) & (sl < (w + 1) * Q)
            if q < nseg:
                inw &= sl < p.S[q]
            dv = np.full(128, -1.0, dtype=np.float32)
            vv = np.zeros(128, dtype=np.float32)
            gsl = sl[inw]
            dv[inw] = sl_doff[q][gsl]
            vv[inw] = sl_dis[q][gsl]
            doff_cols[:, i] = dv.astype(f16)
            disv_cols[:, i] = vv.astype(f16)
        p.idx_all.append(idx_cols)
        p.doff_all.append(doff_cols)
        p.disv_all.append(disv_cols)

        sdv = np.zeros((w_win, nwin), dtype=np.float32)
        nodes_per_win = memb.reshape(nwin, w_win)
        for w in range(nwin):
            mm = nodes_per_win[w] >= 0
            sdv[mm, w] = (
                p.coef * C_U * dis[c * npc + nodes_per_win[w][mm]]
            ).astype(np.float32)
        p.sd.append(sdv)

    # iota-expanded constant [128, w_win, max colsmax] f16
    cm = max(p.colsmax)
    io = np.broadcast_to(
        np.arange(w_win, dtype=np.float32)[None, :, None], (128, w_win, cm)
    )
    p.iota = np.ascontiguousarray(io.astype(f16))
    p.iota_cm = cm
    p.WT = np.ascontiguousarray(np.asarray(W, dtype=np.float32).T)
    p.b = np.asarray(b, dtype=np.float32)
    p.bias_nonzero = bool(np.any(p.b != 0))
    if p.bias_nonzero:
        sb = np.zeros((n_cores, nwin * w_win), dtype=np.float32)
        np.add.at(sb, (core_of, e_w * w_win + e_off), dis[src])
        for c in range(n_cores):
            memb = p.win_members[c]
            real = memb >= 0
            slots = np.arange(nwin * w_win)[real]
            sb[c, slots] += dis[c * npc + memb[real]]
        p.sb = sb.reshape(n_cores, 1, nwin * w_win)
    return p


def build_program(p, gbufs=10, scratch=32768):
    import concourse.bacc as bacc
    import concourse.mybir as mybir
    import concourse.tile as tile

    f32, f16i, i16 = mybir.dt.float32, mybir.dt.float16, mybir.dt.int16
    D, nwin, group = p.D, p.nwin, p.group
    nseg = p.nseg

    nc = bacc.Bacc(
        "TRN2",
        target_bir_lowering=False,
        debug=False,
        num_swdge_queues=4,
        dynamic_dma_scratch_size=scratch,
    )
    x_d = nc.dram_tensor("x", [p.N, D], f16i, kind="ExternalInput")
    xself_d = nc.dram_tensor(
        "xself", [128, p.nwin * p.w_win // 128, D], f16i, kind="ExternalInput"
    )
    wt_d = nc.dram_tensor("wt", [D, D], f32, kind="ExternalInput")
    iota_d = nc.dram_tensor(
        "iota", [128, p.w_win, p.iota_cm], f16i, kind="ExternalInput"
    )
    idx_d = nc.dram_tensor("idx", [128, p.idx_cols_tot], i16, kind="ExternalInput")
    doff_d = nc.dram_tensor("doff", [128, p.tot_pass], f16i, kind="ExternalInput")
    disv_d = nc.dram_tensor("disv", [128, p.tot_pass], f16i, kind="ExternalInput")
    sd_d = nc.dram_tensor("sd", [p.w_win, nwin], f32, kind="ExternalInput")
    if p.bias_nonzero:
        sb_d = nc.dram_tensor("sb", [1, nwin * p.w_win], f32, kind="ExternalInput")
        b_d = nc.dram_tensor("b", [1, D], f32, kind="ExternalInput")
    out_d = nc.dram_tensor("out", [p.w_win, nwin, D], f16i, kind="ExternalOutput")

    segs = []
    for q in range(nseg):
        lo = q * SEG
        hi = min(p.N, lo + SEG)
        segs.append(x_d[lo:hi, :])

    # per-stream idx base col (16-wrapped)
    idx_base = []
    ib = 0
    for q in range(nseg):
        idx_base.append(ib)
        ib += p.P[q] // 16

    # vh metadata base per (g, q)
    vh_base = {}
    pb = 0
    for g in range(p.ngroups):
        for q in range(nseg + 1):
            vh_base[(g, q)] = pb
            pb += p.gq_npas[g][q]
    assert pb == p.tot_pass

    with tile.TileContext(nc) as tc:
        with (
            tc.tile_pool(name="const", bufs=1) as constp,
            tc.tile_pool(name="gbuf", bufs=1) as gbufp,
            tc.tile_pool(name="vh", bufs=2) as vhp,
            tc.tile_pool(name="aggx", bufs=3) as aggxp,
            tc.tile_pool(name="outsb", bufs=2) as outp,
            tc.tile_pool(name="ps1", bufs=4, space="PSUM") as ps1p,
            tc.tile_pool(name="ps2", bufs=2, space="PSUM") as ps2p,
        ):
            # idx: separate tiles per (stream, quarter) so the first gathers
            # only wait on the first ~800KB slice, not the whole 3.3MB table.
            # Slice boundaries are multiples of 64 cols (= one 1024-idx call)
            # so a call's idx slice never straddles tiles.
            idx_tiles = []  # per stream: list of (start_col, ncols, tile)
            for q in range(nseg):
                cq = p.P[q] // 16
                sl = _ceil(_ceil(cq, 4), 64) * 64
                tl = []
                for s in range(0, cq, sl):
                    e = min(cq, s + sl)
                    t = constp.tile([128, e - s], i16, tag=f"idx{q}_{s // sl}")
                    tl.append((s, e - s, t))
                idx_tiles.append(tl)
            idx_loaded = set()

            def _load_idx(q, k):
                if k >= len(idx_tiles[q]) or (q, k) in idx_loaded:
                    return
                idx_loaded.add((q, k))
                a, n, t = idx_tiles[q][k]
                nc.sync.dma_start(
                    t[:], idx_d[:, idx_base[q] + a : idx_base[q] + a + n]
                )

            # interleaved per-stream slices, head of each stream first
            for s in range(4):
                for q in range(nseg):
                    _load_idx(q, s)
            # meta tables go on the Scalar engine's HWDGE queue so they
            # don't queue behind the idx loads. iota + the first doff/disv
            # quarter go first so DVE can start building vh immediately.
            iota_sb = constp.tile([128, p.w_win, p.iota_cm], f16i, tag="iota")
            nc.scalar.dma_start(iota_sb[:], iota_d[:])
            gquart = _ceil(p.ngroups, 4)
            meta_tiles = []  # (pass_start, npas, doff_tile, disv_tile)
            for s in range(0, p.ngroups, gquart):
                ge = min(p.ngroups, s + gquart)
                a = vh_base[(s, 0)]
                e = vh_base[(ge, 0)] if ge < p.ngroups else p.tot_pass
                dt_ = constp.tile([128, e - a], f16i, tag=f"doff{s}")
                vt_ = constp.tile([128, e - a], f16i, tag=f"disv{s}")
                nc.scalar.dma_start(dt_[:], doff_d[:, a:e])
                nc.scalar.dma_start(vt_[:], disv_d[:, a:e])
                meta_tiles.append((a, e - a, dt_, vt_))
            wt32 = constp.tile([D, D], f32, tag="wt32")
            nc.scalar.dma_start(wt32[:], wt_d[:])
            wt16 = constp.tile([D, D], f16i, tag="wt16")
            nc.scalar.copy(wt16[:], wt32[:])
            sd_sb = constp.tile([p.w_win, nwin], f32, tag="sd")
            nc.scalar.dma_start(sd_sb[:], sd_d[:])
            # xself quarters: only the first loads at startup; the rest are
            # emitted inside the group loop (2 groups ahead of first use) to
            # keep the startup DMA burst off the early gather batches.
            xcols = p.nwin * p.w_win // 128
            xquart = _ceil(_ceil(xcols, 4), 1)
            xself_tiles = []  # (col_start, ncols, tile)
            xself_emit_at = {}  # group -> list of quarter indices
            for si, s in enumerate(range(0, xcols, xquart)):
                e = min(xcols, s + xquart)
                xt = constp.tile([128, e - s, D], f16i, tag=f"xself{s}")
                xself_tiles.append((s, e - s, xt))
                if si == 0:
                    nc.scalar.dma_start(xt[:], xself_d[:, s:e, :])
                else:
                    # first window using col s is 2*s; its group minus 2
                    g_emit = max(0, (2 * s) // group - 2)
                    xself_emit_at.setdefault(g_emit, []).append(si)

            def _meta(base, npas):
                for a, n, dt_, vt_ in meta_tiles:
                    if a <= base and base + npas <= a + n:
                        return dt_[:, base - a : base - a + npas], vt_[
                            :, base - a : base - a + npas
                        ]
                raise AssertionError((base, npas))

            def _xself(col):
                for s, n, xt in xself_tiles:
                    if s <= col < s + n:
                        return xt[:, col - s, :]
                raise AssertionError(col)
            if p.bias_nonzero:
                sb_sb = constp.tile([1, nwin * p.w_win], f32, tag="sb")
                nc.sync.dma_start(sb_sb[:], sb_d[:])
                b32 = constp.tile([1, D], f32, tag="b32")
                nc.sync.dma_start(b32[:], b_d[:])
                b16 = constp.tile([1, D], f16i, tag="b16")
                nc.scalar.copy(b16[:], b32[:])
                sbrow16 = constp.tile([1, nwin * p.w_win], f16i, tag="sbw16")
                nc.scalar.copy(sbrow16[:], sb_sb[:])

            # gather calls: per-call tiles, issued in window-progress order
            gtiles = [{} for _ in range(nseg)]  # q -> {tile_index: handle}
            rot = [gbufs if p.P[q] > 8 * CALL else 2 for q in range(nseg)]
            for ci, (q, st, n) in enumerate(p.calls):
                t = st // CALL
                if t not in gtiles[q]:
                    gt = gbufp.tile(
                        [128, TILE_COLS, D], f16i, tag=f"g{q}_{t % rot[q]}"
                    )
                    gtiles[q][t] = gt
                gt = gtiles[q][t]
                lo = (st % CALL) // 128
                c0 = st // 16
                islice = None
                for k, (a, ncols, it) in enumerate(idx_tiles[q]):
                    if a <= c0 < a + ncols:
                        islice = it[:, c0 - a : c0 - a + n // 16]
                        break
                nc.gpsimd.dma_gather(
                    gt[:, lo : lo + n // 128, :],
                    segs[q],
                    islice,
                    n,
                    n,
                    D,
                    queue_num=ci % 4,
                )

            # vh builds + window passes, group-major
            for g, gs in enumerate(p.group_sizes):
                w0 = g * group
                for si in xself_emit_at.get(g, []):
                    s, ncx, xt = xself_tiles[si]
                    nc.scalar.dma_start(xt[:], xself_d[:, s : s + ncx, :])
                vts = []
                for q in range(nseg + 1):
                    npas = p.gq_npas[g][q]
                    base = vh_base[(g, q)]
                    vt = vhp.tile([128, p.w_win, p.colsmax[q]], f16i, tag=f"v{q}")
                    doff_ap, disv_ap = _meta(base, npas)

                    def _bcast(ap2d, n=npas):
                        return ap2d.rearrange(
                            "p (o c) -> p o c", o=1
                        ).broadcast_to([128, p.w_win, n])

                    nc.vector.tensor_tensor(
                        vt[:, :, :npas],
                        iota_sb[:, :, :npas],
                        _bcast(doff_ap),
                        mybir.AluOpType.is_equal,
                    )
                    nc.vector.tensor_tensor(
                        vt[:, :, :npas],
                        vt[:, :, :npas],
                        _bcast(disv_ap),
                        mybir.AluOpType.mult,
                    )
                    vts.append(vt)

                out_sb = outp.tile([p.w_win, gs, D], f16i, tag="out")
                pass_ctr = [0] * (nseg + 1)
                for wl in range(gs):
                    w = w0 + wl
                    ps1 = ps1p.tile([D, p.w_win], f32, tag="ps1")
                    plist = p.win_passes[w]
                    for k, (q, col) in enumerate(plist):
                        if q < nseg:
                            ms = gtiles[q][col // TILE_COLS][
                                :, col % TILE_COLS, :
                            ]
                        else:
                            ms = _xself(col)
                        pl = pass_ctr[q]
                        pass_ctr[q] += 1
                        nc.tensor.matmul(
                            ps1[:, :],
                            ms,
                            vts[q][:, :, pl],
                            start=(k == 0),
                            stop=(k == len(plist) - 1),
                        )
                    ag = aggxp.tile([D, p.w_win], f16i, tag="ag")
                    nc.scalar.copy(ag[:], ps1[:])
                    ps2 = ps2p.tile([p.w_win, D], f32, tag="ps2")
                    nc.tensor.matmul(
                        ps2[:, :],
                        ag[:, :],
                        wt16[:, :],
                        start=True,
                        stop=not p.bias_nonzero,
                    )
                    if p.bias_nonzero:
                        nc.tensor.matmul(
                            ps2[:, :],
                            sbrow16[:, w * p.w_win : (w + 1) * p.w_win],
                            b16[:, :],
                            start=False,
                            stop=True,
                        )
                    nc.scalar.activation(
                        out_sb[:, wl, :],
                        ps2[:, :],
                        mybir.ActivationFunctionType.Relu,
                        scale=sd_sb[:, w : w + 1],
                    )
                nc.sync.dma_start(out_d[:, w0 : w0 + gs, :], out_sb[:])
    nc.compile()
    return nc


def _unshard(p, outs):
    N, D = p.N, p.D
    res = np.empty((N, D), dtype=np.float32)
    for c in range(p.n_cores):
        o = (
            np.asarray(outs[c])
            .astype(np.float32)
            .transpose(1, 0, 2)
            .reshape(p.nwin * p.w_win, D)
        )
        memb = p.win_members[c]
        real = memb >= 0
        res[c * p.npc + memb[real]] = o[real]
    return res


def _in_maps(p):
    maps = []
    for c in range(p.n_cores):
        m = {
            "x": p.x16,
            "xself": p.xself_perm[c],
            "wt": p.WT,
            "iota": p.iota,
            "idx": p.idx_all[c],
            "doff": p.doff_all[c],
            "disv": p.disv_all[c],
            "sd": p.sd[c],
        }
        if p.bias_nonzero:
            m["sb"] = p.sb[c]
            m["b"] = p.b.reshape(1, -1)
        maps.append(m)
    return maps


def kernel(x, edge_index, W, b):
    from concourse.bass_utils import run_bass_kernel_spmd

    x = np.asarray(x, dtype=np.float32)
    W = np.asarray(W, dtype=np.float32)
    b = np.asarray(b, dtype=np.float32)
    p = prepare(x, edge_index, W, b)
    nc = build_program(p)
    res = run_bass_kernel_spmd(nc, _in_maps(p), core_ids=list(range(p.n_cores)))
    outs = [r["out"] for r in res.results]
    return _unshard(p, outs)
